# revision 7
# baseline (speedup 1.0000x reference)
"""GQA kernel for trn2, 8 NeuronCores.

Sharding: core c = (b, g2) with b = c//4, g2 = c%4.  Each core handles batch b
and kv heads {2*g2, 2*g2+1} (q heads 8*g2 .. 8*g2+7).  Wq/Wk/Wv column-sharded,
Wo row-sharded; host sums the 4 partial y outputs per batch (row-shard unshard).

Device dataflow (everything in "transposed" orientation so no x/q/k transposes
are ever needed on-device):
  qT[e,s] = sum_d Wq[d,e] * xT[d,s]     (Wq stationary, xT moving, full-rate f32r)
  kT likewise; vT likewise then PE-transposed to natural v[s,e].
  RoPE applied to qT/kT tiles via partition-shift DMAs + host-baked cos/sin.
  scoresT[j,i] = sum_e kT[e,j] * qT[e,i]  (two heads packed in array rows 0-63/64-127)
  expT = exp(scoresT/8) via ACT, causal-masked by DVE multiply on diagonal tiles
  outT[e,i] (+ rowsum in row 64) = sum_j v_ones[j,e|1] * expT[j,i]
  normalize by broadcasted 1/rowsum, then y[s,d] = sum_f outT[f,s] * Wo[f,d].

Q heads within a core are permuted [0,4,1,5,2,6,3,7] so that each qT partition
tile pairs one kv0-head (rows 0-63) with one kv1-head (rows 64-127), matching
kT's natural kv0|kv1 partition layout.  Wo rows are permuted identically.
"""

import os
import numpy as np

import concourse.bass as bass
import concourse.bacc as bacc
import concourse.mybir as mybir
import concourse.tile as tile
from concourse.bass_utils import run_bass_kernel_spmd

F32 = mybir.dt.float32
F32R = mybir.dt.float32r

B, S, D = 2, 2048, 2048
H, KV, HD = 32, 8, 64
N_CORES = 8
SB = 512          # s-block width (moving free dim)
NSB = S // SB     # 4
NDT = D // 128    # 16 d-tiles
NET = 4           # q e-tiles per core (512 q-cols / 128)
NIT = S // SB     # 4 i-blocks
NJT = S // 128    # 16 j-tiles
PERM = [0, 4, 1, 5, 2, 6, 3, 7]

LAST_RESULT = None  # test.py reads exec_time_ns off this


def _r(ap):
    return ap.bitcast(F32R)


def build_nc():
    nc = bacc.Bacc("TRN2", target_bir_lowering=False, debug=False,
                   enable_asserts=True, num_devices=N_CORES)

    xT = nc.dram_tensor("xT", [D, S], F32R, kind="ExternalInput")
    wq = nc.dram_tensor("wq", [D, 512], F32R, kind="ExternalInput")
    wk = nc.dram_tensor("wk", [D, 128], F32R, kind="ExternalInput")
    wv = nc.dram_tensor("wv", [D, 128], F32R, kind="ExternalInput")
    wo = nc.dram_tensor("wo", [512, D], F32R, kind="ExternalInput")
    cos2 = nc.dram_tensor("cos2", [128, S], F32, kind="ExternalInput")
    sin2 = nc.dram_tensor("sin2", [128, S], F32, kind="ExternalInput")
    cmask = nc.dram_tensor("cmask", [128, 4, SB], F32, kind="ExternalInput")
    ident = nc.dram_tensor("ident", [128, 128], F32, kind="ExternalInput")
    y = nc.dram_tensor("y", [S, D], F32, kind="ExternalOutput")
    rscratch = nc.dram_tensor("rscratch", [32, SB], F32)  # internal

    with tile.TileContext(nc) as tc:
        with (
            tc.tile_pool(name="persist", bufs=1) as persist,
            tc.tile_pool(name="consts", bufs=1) as consts,
        ):
            # ---- persistent SBUF tensors ----
            qT_sb = [persist.tile([128, S], F32R, name=f"qT{t}") for t in range(NET)]
            kT_sb = persist.tile([128, S], F32R, name="kT")
            v_ones0 = persist.tile([128, NJT, 65], F32R, name="v_ones0")
            v_ones1 = persist.tile([128, NJT, 65], F32R, name="v_ones1")
            outT = [persist.tile([128, S], F32R, name=f"outT{t}") for t in range(NET)]
            rowsum = persist.tile([32, SB], F32, name="rowsum")
            recip = persist.tile([32, SB], F32, name="recip")

            cos_sb = consts.tile([128, S], F32, name="cos_sb")
            sin_sb = consts.tile([128, S], F32, name="sin_sb")
            cmask_sb = consts.tile([128, 4, SB], F32, name="cmask_sb")
            ident_sb = consts.tile([128, 128], F32, name="ident_sb")
            nc.sync.dma_start(cos_sb[:], cos2[:])
            nc.sync.dma_start(sin_sb[:], sin2[:])
            nc.sync.dma_start(cmask_sb[:], cmask[:])
            nc.sync.dma_start(ident_sb[:], ident[:])
            ones_col = consts.tile([128, NJT, 1], F32, name="ones_col")
            nc.gpsimd.memset(ones_col[:], 1.0)
            nc.vector.tensor_copy(v_ones0[:, :, 64:65], ones_col[:])
            nc.vector.tensor_copy(v_ones1[:, :, 64:65], ones_col[:])

            # ================= Phase A: projections =================
            with (
                tc.tile_pool(name="wpool", bufs=1) as wpool,
                tc.tile_pool(name="xpool", bufs=3) as xpool,
                tc.tile_pool(name="apsum", bufs=1, space="PSUM") as apsum,
                tc.tile_pool(name="trpsum", bufs=2, space="PSUM") as trpsum,
                tc.tile_pool(name="atmp", bufs=3) as atmp,
            ):
                wq_sb = [wpool.tile([128, 512], F32R, name=f"wq{d}") for d in range(NDT)]
                wk_sb = [wpool.tile([128, 128], F32R, name=f"wk{d}") for d in range(NDT)]
                wv_sb = [wpool.tile([128, 128], F32R, name=f"wv{d}") for d in range(NDT)]
                for d in range(NDT):
                    nc.sync.dma_start(wq_sb[d][:], wq[d * 128:(d + 1) * 128, :])
                    nc.sync.dma_start(wk_sb[d][:], wk[d * 128:(d + 1) * 128, :])
                    nc.sync.dma_start(wv_sb[d][:], wv[d * 128:(d + 1) * 128, :])

                for sb in range(NSB):
                    scol = slice(sb * SB, (sb + 1) * SB)
                    # six accumulating psum tiles live across the d loop
                    ps_q = [apsum.tile([128, SB], F32, name=f"psq{t}", tag=f"psq{t}")
                            for t in range(NET)]
                    ps_k = apsum.tile([128, SB], F32, name="psk", tag="psk")
                    ps_v = apsum.tile([128, SB], F32, name="psv", tag="psv")
                    for d in range(NDT):
                        xt = xpool.tile([128, SB], F32R, name="xt", tag="xt")
                        nc.sync.dma_start(xt[:], xT[d * 128:(d + 1) * 128, scol])
                        st, sp = d == 0, d == NDT - 1
                        for t in range(NET):
                            nc.tensor.matmul(ps_q[t][:], _r(wq_sb[d][:, t * 128:(t + 1) * 128]),
                                             _r(xt[:]), start=st, stop=sp)
                        nc.tensor.matmul(ps_k[:], _r(wk_sb[d][:]), _r(xt[:]), start=st, stop=sp)
                        nc.tensor.matmul(ps_v[:], _r(wv_sb[d][:]), _r(xt[:]), start=st, stop=sp)

                    # RoPE on q tiles and k tile: dst = ps*cos + shift(ps)*sin
                    # (DMA cannot read PSUM, so bounce through SBUF first)
                    for ps, dst in [(ps_q[t], qT_sb[t]) for t in range(NET)] + [(ps_k, kT_sb)]:
                        qtmp = atmp.tile([128, SB], F32, name="qtmp", tag="qtmp")
                        nc.vector.tensor_copy(qtmp[:], ps[:])
                        rot = atmp.tile([128, SB], F32, name="rot", tag="rot")
                        for (a, b_) in ((0, 32), (32, 0), (64, 96), (96, 64)):
                            nc.sync.dma_start(rot[b_:b_ + 32, :], qtmp[a:a + 32, :])
                        t1 = atmp.tile([128, SB], F32, name="t1", tag="t1")
                        nc.vector.tensor_mul(t1[:], qtmp[:], cos_sb[:, scol])
                        t2 = atmp.tile([128, SB], F32, name="t2", tag="t2")
                        nc.vector.tensor_mul(t2[:], rot[:], sin_sb[:, scol])
                        nc.vector.tensor_add(dst[:, scol], t1[:], t2[:])

                    # v: copy psum -> sbuf, PE-transpose 128x128 blocks, split kv heads
                    vtmp = atmp.tile([128, SB], F32, name="vtmp", tag="vtmp")
                    nc.vector.tensor_copy(vtmp[:], ps_v[:])
                    for u in range(SB // 128):
                        jt = sb * 4 + u
                        tr = trpsum.tile([128, 128], F32, name="tr", tag="tr")
                        nc.tensor.transpose(tr[:], vtmp[:, u * 128:(u + 1) * 128], ident_sb[:])
                        nc.vector.tensor_copy(v_ones0[:, jt, 0:64], tr[:, 0:64])
                        nc.vector.tensor_copy(v_ones1[:, jt, 0:64], tr[:, 64:128])

            # ================= Phase B: attention =================
            with (
                tc.tile_pool(name="scpsum", bufs=2, space="PSUM") as scpsum,
                tc.tile_pool(name="pvpsum", bufs=1, space="PSUM") as pvpsum,
                tc.tile_pool(name="epool", bufs=4) as epool,
                tc.tile_pool(name="btmp", bufs=3) as btmp,
            ):
                for t in range(NET):
                    for bi in range(NIT):
                        icol = slice(bi * SB, (bi + 1) * SB)
                        pvA = pvpsum.tile([65, SB], F32, name="pvA", tag="pvA")
                        pvB = pvpsum.tile([65, SB], F32, name="pvB", tag="pvB")
                        njt = 4 * bi + 4
                        for jt in range(njt):
                            jcol = slice(jt * 128, (jt + 1) * 128)
                            scA = scpsum.tile([128, SB], F32, name="scA", tag="scA")
                            scB = scpsum.tile([128, SB], F32, name="scB", tag="scB")
                            nc.tensor.matmul(scA[:], _r(kT_sb[0:64, jcol]),
                                             _r(qT_sb[t][0:64, icol]), start=True, stop=True)
                            nc.tensor.matmul(scB[:], _r(kT_sb[64:128, jcol]),
                                             _r(qT_sb[t][64:128, icol]), start=True, stop=True)
                            eA = epool.tile([128, SB], F32R, name="eA", tag="eA")
                            eB = epool.tile([128, SB], F32R, name="eB", tag="eB")
                            nc.scalar.activation(eA[:], scA[:],
                                                 mybir.ActivationFunctionType.Exp, scale=0.125)
                            nc.scalar.activation(eB[:], scB[:],
                                                 mybir.ActivationFunctionType.Exp, scale=0.125)
                            if jt >= 4 * bi:
                                ro = jt - 4 * bi
                                nc.vector.tensor_mul(eA[:], eA[:], cmask_sb[:, ro, :])
                                nc.vector.tensor_mul(eB[:], eB[:], cmask_sb[:, ro, :])
                            st, sp = jt == 0, jt == njt - 1
                            nc.tensor.matmul(pvA[:], _r(v_ones0[:, jt, :]), _r(eA[:]),
                                             start=st, stop=sp)
                            nc.tensor.matmul(pvB[:], _r(v_ones1[:, jt, :]), _r(eB[:]),
                                             start=st, stop=sp)
                        # unnormalized outT + rowsum extraction (PSUM -> SBUF via
                        # DVE, then DMA for the cross-partition moves)
                        nc.vector.tensor_copy(outT[t][0:64, icol], pvA[0:64, :])
                        pvsB = btmp.tile([65, SB], F32R, name="pvsB", tag="pvsB")
                        nc.vector.tensor_copy(pvsB[:], pvB[:])
                        nc.sync.dma_start(outT[t][64:128, icol], pvsB[0:64, :])
                        rowA = btmp.tile([65, SB], F32, name="rowA", tag="rowA")
                        nc.vector.tensor_copy(rowA[64:65, :], pvA[64:65, :])
                        rA = (t * 2 + 0) * 4 + bi
                        rB = (t * 2 + 1) * 4 + bi
                        nc.sync.dma_start(rowsum[rA:rA + 1, :], rowA[64:65, :])
                        nc.sync.dma_start(rowsum[rB:rB + 1, :], pvsB[64:65, :].bitcast(F32))

                # normalize: recip once, broadcast via DRAM bounce, multiply in
                # place.  bc is [128, SB] with each 64-row half holding the
                # head's 1/rowsum so DVE operand start-partitions match.
                nc.vector.reciprocal(recip[:], rowsum[:])
                nc.sync.dma_start(rscratch[:], recip[:])
                for t in range(NET):
                    for bi in range(NIT):
                        rA = (t * 2 + 0) * 4 + bi
                        rB = (t * 2 + 1) * 4 + bi
                        icol = slice(bi * SB, (bi + 1) * SB)
                        bc = btmp.tile([128, SB], F32, name="bc", tag="bc")
                        nc.sync.dma_start(
                            bc[0:64, :], rscratch[rA:rA + 1, :].broadcast_to((64, SB)))
                        nc.sync.dma_start(
                            bc[64:128, :], rscratch[rB:rB + 1, :].broadcast_to((64, SB)))
                        nc.vector.tensor_mul(outT[t][0:64, icol],
                                             outT[t][0:64, icol], bc[0:64, :])
                        nc.vector.tensor_mul(outT[t][64:128, icol],
                                             outT[t][64:128, icol], bc[64:128, :])

            # ================= Phase C: output projection =================
            with (
                tc.tile_pool(name="wopool", bufs=1) as wopool,
                tc.tile_pool(name="ypsum", bufs=2, space="PSUM") as ypsum,
                tc.tile_pool(name="ypool", bufs=3) as ypool,
            ):
                wo_sb = [wopool.tile([128, D], F32R, name=f"wo{f}") for f in range(4)]
                for f in range(4):
                    nc.sync.dma_start(wo_sb[f][:], wo[f * 128:(f + 1) * 128, :])
                for stt in range(S // 128):
                    srow = slice(stt * 128, (stt + 1) * 128)
                    for db in range(D // SB):
                        dcol = slice(db * SB, (db + 1) * SB)
                        yp = ypsum.tile([128, SB], F32, name="yp", tag="yp")
                        for f in range(4):
                            nc.tensor.matmul(yp[:], _r(outT[f][:, srow]),
                                             _r(wo_sb[f][:, dcol]),
                                             start=(f == 0), stop=(f == 3))
                        ys = ypool.tile([128, SB], F32, name="ys", tag="ys")
                        nc.vector.tensor_copy(ys[:], yp[:])
                        nc.sync.dma_start(y[srow, dcol], ys[:])

    nc.compile()
    return nc


def host_inputs(x, Wq, Wk, Wv, Wo):
    """Per-core input maps (8 cores)."""
    inv = 1.0 / (10000.0 ** (np.arange(0, HD, 2, dtype=np.float64) / HD))
    freqs = np.outer(np.arange(S, dtype=np.float64), inv)          # [S, 32]
    emb = np.concatenate([freqs, freqs], axis=1)                   # [S, 64]
    cos = np.cos(emb).astype(np.float32)
    sin = np.sin(emb).astype(np.float32)
    cos2 = np.ascontiguousarray(np.tile(cos.T, (2, 1)))            # [128, S]
    sinf = np.concatenate([-sin[:, :32], sin[:, 32:]], axis=1)     # sign-folded
    sin2 = np.ascontiguousarray(np.tile(sinf.T, (2, 1)))
    j = np.arange(128)[:, None, None]
    ro = np.arange(4)[None, :, None]
    i = np.arange(SB)[None, None, :]
    cmask = (j + 128 * ro <= i).astype(np.float32)                 # [128, 4, 512]
    ident = np.eye(128, dtype=np.float32)

    Wq4 = Wq.reshape(D, H, HD)
    Wo4 = Wo.reshape(H, HD, D)
    Wk4 = Wk.reshape(D, KV, HD)
    Wv4 = Wv.reshape(D, KV, HD)

    maps = []
    for c in range(N_CORES):
        b, g2 = c // 4, c % 4
        gh = [8 * g2 + p for p in PERM]
        maps.append({
            "xT": np.ascontiguousarray(x[b].T),
            "wq": np.ascontiguousarray(Wq4[:, gh, :].reshape(D, 512)),
            "wk": np.ascontiguousarray(Wk4[:, [2 * g2, 2 * g2 + 1], :].reshape(D, 128)),
            "wv": np.ascontiguousarray(Wv4[:, [2 * g2, 2 * g2 + 1], :].reshape(D, 128)),
            "wo": np.ascontiguousarray(Wo4[gh].reshape(512, D)),
            "cos2": cos2, "sin2": sin2, "cmask": cmask, "ident": ident,
        })
    return maps


_NC_CACHE = None


def kernel(x, Wq, Wk, Wv, Wo):
    global LAST_RESULT, _NC_CACHE
    x = np.asarray(x, np.float32)
    maps = host_inputs(np.asarray(x, np.float32), np.asarray(Wq, np.float32),
                       np.asarray(Wk, np.float32), np.asarray(Wv, np.float32),
                       np.asarray(Wo, np.float32))
    if _NC_CACHE is None:
        _NC_CACHE = build_nc()
    trace = bool(os.environ.get("KERNEL_TRACE"))
    try:
        res = run_bass_kernel_spmd(_NC_CACHE, maps, list(range(N_CORES)), trace=trace)
    except (ImportError, ModuleNotFoundError):
        # NTFF profile hook unavailable in this environment — run untraced.
        res = run_bass_kernel_spmd(_NC_CACHE, maps, list(range(N_CORES)), trace=False)
    LAST_RESULT = res
    out = np.empty((B, S, D), np.float32)
    for b in range(B):
        out[b] = res.results[4 * b]["y"]
        for g2 in range(1, 4):
            out[b] += res.results[4 * b + g2]["y"]
    return out


# revision 13
# speedup vs baseline: 1.2167x; 1.2167x over previous
"""GQA kernel for trn2, 8 NeuronCores.

Sharding: core c = (b, g2) with b = c//4, g2 = c%4.  Each core handles batch b
and kv heads {2*g2, 2*g2+1} (q heads 8*g2 .. 8*g2+7).  Wq/Wk/Wv column-sharded,
Wo row-sharded; host sums the 4 partial y outputs per batch (row-shard unshard).

Device dataflow (everything in "transposed" orientation so no x/q/k transposes
are ever needed on-device):
  qT[e,s] = sum_d Wq[d,e] * xT[d,s]     (Wq stationary, xT moving, full-rate f32r)
  kT likewise; vT likewise then PE-transposed to natural v[s,e].
  RoPE applied to qT/kT tiles via partition-shift DMAs + host-baked cos/sin.
  scoresT[j,i] = sum_e kT[e,j] * qT[e,i]  (two heads packed in array rows 0-63/64-127)
  expT = exp(scoresT/8) via ACT, causal-masked by DVE multiply on diagonal tiles
  outT[e,i] (+ rowsum in row 64) = sum_j v_ones[j,e|1] * expT[j,i]
  normalize by broadcasted 1/rowsum, then y[s,d] = sum_f outT[f,s] * Wo[f,d].

Q heads within a core are permuted [0,4,1,5,2,6,3,7] so that each qT partition
tile pairs one kv0-head (rows 0-63) with one kv1-head (rows 64-127), matching
kT's natural kv0|kv1 partition layout.  Wo rows are permuted identically.
"""

import os
import numpy as np

import concourse.bass as bass
import concourse.bacc as bacc
import concourse.mybir as mybir
import concourse.tile as tile
from concourse.bass_utils import run_bass_kernel_spmd

F32 = mybir.dt.float32
F32R = mybir.dt.float32r

B, S, D = 2, 2048, 2048
H, KV, HD = 32, 8, 64
N_CORES = 8
SB = 512          # s-block width (moving free dim)
NSB = S // SB     # 4
NDT = D // 128    # 16 d-tiles
NET = 4           # q e-tiles per core (512 q-cols / 128)
NIT = S // SB     # 4 i-blocks
NJT = S // 128    # 16 j-tiles
PERM = [0, 4, 1, 5, 2, 6, 3, 7]

LAST_RESULT = None  # test.py reads exec_time_ns off this


def _r(ap):
    return ap.bitcast(F32R)


def build_nc():
    nc = bacc.Bacc("TRN2", target_bir_lowering=False, debug=False,
                   enable_asserts=True, num_devices=N_CORES)

    xT = nc.dram_tensor("xT", [D, S], F32R, kind="ExternalInput")
    wq = nc.dram_tensor("wq", [D, 512], F32R, kind="ExternalInput")
    wk = nc.dram_tensor("wk", [D, 128], F32R, kind="ExternalInput")
    wv = nc.dram_tensor("wv", [D, 128], F32R, kind="ExternalInput")
    wo = nc.dram_tensor("wo", [512, D], F32R, kind="ExternalInput")
    cos2 = nc.dram_tensor("cos2", [128, S], F32, kind="ExternalInput")
    sin2 = nc.dram_tensor("sin2", [128, S], F32, kind="ExternalInput")
    cmask = nc.dram_tensor("cmask", [128, 4, SB], F32, kind="ExternalInput")
    ident = nc.dram_tensor("ident", [128, 128], F32, kind="ExternalInput")
    y = nc.dram_tensor("y", [S, D], F32, kind="ExternalOutput")
    rscratch = nc.dram_tensor("rscratch", [NIT, 8, SB], F32)  # internal

    with tile.TileContext(nc) as tc:
        with (
            tc.tile_pool(name="persist", bufs=1) as persist,
            tc.tile_pool(name="consts", bufs=1) as consts,
        ):
            # ---- persistent SBUF tensors ----
            qT_sb = [persist.tile([128, S], F32R, name=f"qT{t}") for t in range(NET)]
            kT_sb = persist.tile([128, S], F32R, name="kT")
            v_ones0 = persist.tile([128, NJT, 65], F32R, name="v_ones0")
            v_ones1 = persist.tile([128, NJT, 65], F32R, name="v_ones1")
            outT = [persist.tile([128, S], F32R, name=f"outT{t}") for t in range(NET)]
            rowsum = [persist.tile([8, SB], F32, name=f"rowsum{b_}") for b_ in range(NIT)]
            recip = [persist.tile([8, SB], F32, name=f"recip{b_}") for b_ in range(NIT)]

            cos_sb = consts.tile([128, S], F32, name="cos_sb")
            sin_sb = consts.tile([128, S], F32, name="sin_sb")
            cmask_sb = consts.tile([128, 4, SB], F32, name="cmask_sb")
            ident_sb = consts.tile([128, 128], F32, name="ident_sb")
            ones_col = consts.tile([128, NJT, 1], F32, name="ones_col")
            nc.gpsimd.memset(ones_col[:], 1.0)
            nc.vector.tensor_copy(v_ones0[:, :, 64:65], ones_col[:])
            nc.vector.tensor_copy(v_ones1[:, :, 64:65], ones_col[:])

            # ================= Phase A: projections =================
            with (
                tc.tile_pool(name="wpool", bufs=1) as wpool,
                tc.tile_pool(name="xpool", bufs=3) as xpool,
                tc.tile_pool(name="apsum", bufs=1, space="PSUM") as apsum,
                tc.tile_pool(name="trpsum", bufs=2, space="PSUM") as trpsum,
                tc.tile_pool(name="atmp", bufs=3) as atmp,
            ):
                wq_sb = [wpool.tile([128, 512], F32R, name=f"wq{d}") for d in range(NDT)]
                wk_sb = [wpool.tile([128, 128], F32R, name=f"wk{d}") for d in range(NDT)]
                wv_sb = [wpool.tile([128, 128], F32R, name=f"wv{d}") for d in range(NDT)]

                for sb in range(NSB):
                    scol = slice(sb * SB, (sb + 1) * SB)
                    # six accumulating psum tiles live across the d loop
                    ps_q = [apsum.tile([128, SB], F32, name=f"psq{t}", tag=f"psq{t}")
                            for t in range(NET)]
                    ps_k = apsum.tile([128, SB], F32, name="psk", tag="psk")
                    ps_v = apsum.tile([128, SB], F32, name="psv", tag="psv")
                    for d in range(NDT):
                        if sb == 0:
                            # first-use weight loads, interleaved with the x stream
                            nc.sync.dma_start(wq_sb[d][:], wq[d * 128:(d + 1) * 128, :])
                            nc.sync.dma_start(wk_sb[d][:], wk[d * 128:(d + 1) * 128, :])
                            nc.sync.dma_start(wv_sb[d][:], wv[d * 128:(d + 1) * 128, :])
                            if d == 10:
                                nc.sync.dma_start(cos_sb[:], cos2[:])
                                nc.sync.dma_start(sin_sb[:], sin2[:])
                            if d == 14:
                                nc.sync.dma_start(ident_sb[:], ident[:])
                        if sb == 1 and d == 4:
                            nc.sync.dma_start(cmask_sb[:], cmask[:])
                        xt = xpool.tile([128, SB], F32R, name="xt", tag="xt")
                        nc.sync.dma_start(xt[:], xT[d * 128:(d + 1) * 128, scol])
                        st, sp = d == 0, d == NDT - 1
                        for t in range(NET):
                            nc.tensor.matmul(ps_q[t][:], _r(wq_sb[d][:, t * 128:(t + 1) * 128]),
                                             _r(xt[:]), start=st, stop=sp)
                        nc.tensor.matmul(ps_k[:], _r(wk_sb[d][:]), _r(xt[:]), start=st, stop=sp)
                        nc.tensor.matmul(ps_v[:], _r(wv_sb[d][:]), _r(xt[:]), start=st, stop=sp)

                    # v: copy psum -> sbuf, PE-transpose 128x128 blocks, split kv heads
                    vtmp = atmp.tile([128, SB], F32, name="vtmp", tag="vtmp")
                    nc.scalar.copy(vtmp[:], ps_v[:])
                    for u in range(SB // 128):
                        jt = sb * 4 + u
                        tr = trpsum.tile([128, 128], F32, name="tr", tag="tr")
                        nc.tensor.transpose(tr[:], vtmp[:, u * 128:(u + 1) * 128], ident_sb[:])
                        nc.vector.tensor_copy(v_ones0[:, jt, 0:64], tr[:, 0:64])
                        nc.vector.tensor_copy(v_ones1[:, jt, 0:64], tr[:, 64:128])

                    # RoPE on q tiles and k tile: dst = ps*cos + shift(ps)*sin
                    # (DMA cannot read PSUM, so bounce through SBUF first)
                    for ps, dst in [(ps_k, kT_sb)] + [(ps_q[t], qT_sb[t]) for t in range(NET)]:
                        qtmp = atmp.tile([128, SB], F32, name="qtmp", tag="qtmp")
                        nc.scalar.copy(qtmp[:], ps[:])
                        rot = atmp.tile([128, SB], F32, name="rot", tag="rot")
                        for (a, b_) in ((0, 32), (32, 0), (64, 96), (96, 64)):
                            nc.gpsimd.dma_start(rot[b_:b_ + 32, :], qtmp[a:a + 32, :])
                        t1 = atmp.tile([128, SB], F32, name="t1", tag="t1")
                        nc.vector.tensor_mul(t1[:], qtmp[:], cos_sb[:, scol])
                        t2 = atmp.tile([128, SB], F32, name="t2", tag="t2")
                        nc.vector.tensor_mul(t2[:], rot[:], sin_sb[:, scol])
                        nc.vector.tensor_add(dst[:, scol], t1[:], t2[:])

            # ================= Phase B: attention =================
            # bi-outer so phase C (emitted later) can overlap later bi rounds.
            # Diagonal j-tiles only compute the valid column range [lo:512);
            # the triangular 128-strip is masked with cmask[:,0,0:128].
            with (
                tc.tile_pool(name="wopool", bufs=1) as wopool,
                tc.tile_pool(name="scpsum", bufs=2, space="PSUM") as scpsum,
                tc.tile_pool(name="pvpsum", bufs=1, space="PSUM") as pvpsum,
                tc.tile_pool(name="epool", bufs=4) as epool,
                tc.tile_pool(name="btmp", bufs=3) as btmp,
                tc.tile_pool(name="ypsum", bufs=2, space="PSUM") as ypsum,
                tc.tile_pool(name="ypool", bufs=3) as ypool,
            ):
                # prefetch Wo during attention, one f-tile per bi round
                wo_sb = [wopool.tile([128, D], F32R, name=f"wo{f}") for f in range(4)]

                tri = cmask_sb[:, 0, 0:128]
                for bi in range(NIT):
                    nc.sync.dma_start(wo_sb[bi][:], wo[bi * 128:(bi + 1) * 128, :])
                    icol = slice(bi * SB, (bi + 1) * SB)
                    for t in range(NET):
                        pvA = pvpsum.tile([65, SB], F32, name="pvA", tag="pvA")
                        pvB = pvpsum.tile([65, SB], F32, name="pvB", tag="pvB")
                        njt = 4 * bi + 4
                        for jt in range(njt):
                            jcol = slice(jt * 128, (jt + 1) * 128)
                            ro = jt - 4 * bi
                            lo = 128 * max(ro, 0)
                            iband = slice(bi * SB + lo, (bi + 1) * SB)
                            scA = scpsum.tile([128, SB], F32, name="scA", tag="scA")
                            scB = scpsum.tile([128, SB], F32, name="scB", tag="scB")
                            nc.tensor.matmul(scA[:, lo:], _r(kT_sb[0:64, jcol]),
                                             _r(qT_sb[t][0:64, iband]), start=True, stop=True)
                            nc.tensor.matmul(scB[:, lo:], _r(kT_sb[64:128, jcol]),
                                             _r(qT_sb[t][64:128, iband]), start=True, stop=True)
                            eA = epool.tile([128, SB], F32R, name="eA", tag="eA")
                            eB = epool.tile([128, SB], F32R, name="eB", tag="eB")
                            nc.scalar.activation(eA[:, lo:], scA[:, lo:],
                                                 mybir.ActivationFunctionType.Exp, scale=0.125)
                            nc.scalar.activation(eB[:, lo:], scB[:, lo:],
                                                 mybir.ActivationFunctionType.Exp, scale=0.125)
                            if ro >= 0:
                                nc.vector.tensor_mul(eA[:, lo:lo + 128], eA[:, lo:lo + 128], tri)
                                nc.vector.tensor_mul(eB[:, lo:lo + 128], eB[:, lo:lo + 128], tri)
                            st, sp = jt == 0, jt == njt - 1
                            nc.tensor.matmul(pvA[:, lo:], _r(v_ones0[:, jt, :]), _r(eA[:, lo:]),
                                             start=st, stop=sp)
                            nc.tensor.matmul(pvB[:, lo:], _r(v_ones1[:, jt, :]), _r(eB[:, lo:]),
                                             start=st, stop=sp)
                        # unnormalized outT + rowsum rows (2t, 2t+1) of this bi
                        nc.vector.tensor_copy(outT[t][0:64, icol], pvA[0:64, :])
                        pvsB = btmp.tile([65, SB], F32R, name="pvsB", tag="pvsB")
                        nc.vector.tensor_copy(pvsB[:], pvB[:])
                        nc.gpsimd.dma_start(outT[t][64:128, icol], pvsB[0:64, :])
                        rowA = btmp.tile([65, SB], F32, name="rowA", tag="rowA")
                        nc.vector.tensor_copy(rowA[64:65, :], pvA[64:65, :])
                        nc.gpsimd.dma_start(rowsum[bi][2 * t:2 * t + 1, :], rowA[64:65, :])
                        nc.gpsimd.dma_start(rowsum[bi][2 * t + 1:2 * t + 2, :],
                                            pvsB[64:65, :].bitcast(F32))

                    # per-bi normalization (unblocks phase C rows 4bi..4bi+4)
                    nc.vector.reciprocal(recip[bi][:], rowsum[bi][:])
                    nc.gpsimd.dma_start(rscratch[bi], recip[bi][:])
                    for t in range(NET):
                        bc = btmp.tile([128, SB], F32, name="bc", tag="bc")
                        nc.gpsimd.dma_start(
                            bc[0:64, :],
                            rscratch[bi, 2 * t:2 * t + 1, :].broadcast_to((64, SB)))
                        nc.gpsimd.dma_start(
                            bc[64:128, :],
                            rscratch[bi, 2 * t + 1:2 * t + 2, :].broadcast_to((64, SB)))
                        nc.vector.tensor_mul(outT[t][0:64, icol],
                                             outT[t][0:64, icol], bc[0:64, :])
                        nc.vector.tensor_mul(outT[t][64:128, icol],
                                             outT[t][64:128, icol], bc[64:128, :])

            # ================= Phase C: output projection =================
                for stt in range(S // 128):
                    srow = slice(stt * 128, (stt + 1) * 128)
                    for db in range(D // SB):
                        dcol = slice(db * SB, (db + 1) * SB)
                        yp = ypsum.tile([128, SB], F32, name="yp", tag="yp")
                        for f in range(4):
                            nc.tensor.matmul(yp[:], _r(outT[f][:, srow]),
                                             _r(wo_sb[f][:, dcol]),
                                             start=(f == 0), stop=(f == 3))
                        ys = ypool.tile([128, SB], F32, name="ys", tag="ys")
                        nc.scalar.copy(ys[:], yp[:])
                        nc.sync.dma_start(y[srow, dcol], ys[:])

    nc.compile()
    return nc


def host_inputs(x, Wq, Wk, Wv, Wo):
    """Per-core input maps (8 cores)."""
    inv = 1.0 / (10000.0 ** (np.arange(0, HD, 2, dtype=np.float64) / HD))
    freqs = np.outer(np.arange(S, dtype=np.float64), inv)          # [S, 32]
    emb = np.concatenate([freqs, freqs], axis=1)                   # [S, 64]
    cos = np.cos(emb).astype(np.float32)
    sin = np.sin(emb).astype(np.float32)
    cos2 = np.ascontiguousarray(np.tile(cos.T, (2, 1)))            # [128, S]
    sinf = np.concatenate([-sin[:, :32], sin[:, 32:]], axis=1)     # sign-folded
    sin2 = np.ascontiguousarray(np.tile(sinf.T, (2, 1)))
    j = np.arange(128)[:, None, None]
    ro = np.arange(4)[None, :, None]
    i = np.arange(SB)[None, None, :]
    cmask = (j + 128 * ro <= i).astype(np.float32)                 # [128, 4, 512]
    ident = np.eye(128, dtype=np.float32)

    Wq4 = Wq.reshape(D, H, HD)
    Wo4 = Wo.reshape(H, HD, D)
    Wk4 = Wk.reshape(D, KV, HD)
    Wv4 = Wv.reshape(D, KV, HD)

    maps = []
    for c in range(N_CORES):
        b, g2 = c // 4, c % 4
        gh = [8 * g2 + p for p in PERM]
        maps.append({
            "xT": np.ascontiguousarray(x[b].T),
            "wq": np.ascontiguousarray(Wq4[:, gh, :].reshape(D, 512)),
            "wk": np.ascontiguousarray(Wk4[:, [2 * g2, 2 * g2 + 1], :].reshape(D, 128)),
            "wv": np.ascontiguousarray(Wv4[:, [2 * g2, 2 * g2 + 1], :].reshape(D, 128)),
            "wo": np.ascontiguousarray(Wo4[gh].reshape(512, D)),
            "cos2": cos2, "sin2": sin2, "cmask": cmask, "ident": ident,
        })
    return maps


_NC_CACHE = None


def kernel(x, Wq, Wk, Wv, Wo):
    global LAST_RESULT, _NC_CACHE
    x = np.asarray(x, np.float32)
    maps = host_inputs(np.asarray(x, np.float32), np.asarray(Wq, np.float32),
                       np.asarray(Wk, np.float32), np.asarray(Wv, np.float32),
                       np.asarray(Wo, np.float32))
    if _NC_CACHE is None:
        _NC_CACHE = build_nc()
    trace = bool(os.environ.get("KERNEL_TRACE"))
    try:
        res = run_bass_kernel_spmd(_NC_CACHE, maps, list(range(N_CORES)), trace=trace)
    except (ImportError, ModuleNotFoundError):
        # NTFF profile hook unavailable in this environment — run untraced.
        res = run_bass_kernel_spmd(_NC_CACHE, maps, list(range(N_CORES)), trace=False)
    LAST_RESULT = res
    out = np.empty((B, S, D), np.float32)
    for b in range(B):
        out[b] = res.results[4 * b]["y"]
        for g2 in range(1, 4):
            out[b] += res.results[4 * b + g2]["y"]
    return out


# revision 16
# speedup vs baseline: 1.2464x; 1.0244x over previous
"""GQA kernel for trn2, 8 NeuronCores.

Sharding: core c = (b, g2) with b = c//4, g2 = c%4.  Each core handles batch b
and kv heads {2*g2, 2*g2+1} (q heads 8*g2 .. 8*g2+7).  Wq/Wk/Wv column-sharded,
Wo row-sharded; host sums the 4 partial y outputs per batch (row-shard unshard).

Device dataflow (everything in "transposed" orientation so no x/q/k transposes
are ever needed on-device):
  qT[e,s] = sum_d Wq[d,e] * xT[d,s]     (Wq stationary, xT moving, full-rate f32r)
  kT likewise; vT likewise then PE-transposed to natural v[s,e].
  RoPE applied to qT/kT tiles via partition-shift DMAs + host-baked cos/sin.
  scoresT[j,i] = sum_e kT[e,j] * qT[e,i]  (two heads packed in array rows 0-63/64-127)
  expT = exp(scoresT/8) via ACT, causal-masked by DVE multiply on diagonal tiles
  outT[e,i] (+ rowsum in row 64) = sum_j v_ones[j,e|1] * expT[j,i]
  normalize by broadcasted 1/rowsum, then y[s,d] = sum_f outT[f,s] * Wo[f,d].

Q heads within a core are permuted [0,4,1,5,2,6,3,7] so that each qT partition
tile pairs one kv0-head (rows 0-63) with one kv1-head (rows 64-127), matching
kT's natural kv0|kv1 partition layout.  Wo rows are permuted identically.
"""

import os
import numpy as np

import concourse.bass as bass
import concourse.bacc as bacc
import concourse.mybir as mybir
import concourse.tile as tile
from concourse.bass_utils import run_bass_kernel_spmd

F32 = mybir.dt.float32
F32R = mybir.dt.float32r

B, S, D = 2, 2048, 2048
H, KV, HD = 32, 8, 64
N_CORES = 8
SB = 512          # s-block width (moving free dim)
NSB = S // SB     # 4
NDT = D // 128    # 16 d-tiles
NET = 4           # q e-tiles per core (512 q-cols / 128)
NIT = S // SB     # 4 i-blocks
NJT = S // 128    # 16 j-tiles
PERM = [0, 4, 1, 5, 2, 6, 3, 7]

LAST_RESULT = None  # test.py reads exec_time_ns off this


def _r(ap):
    return ap.bitcast(F32R)


def build_nc():
    nc = bacc.Bacc("TRN2", target_bir_lowering=False, debug=False,
                   enable_asserts=True, num_devices=N_CORES)

    xT = nc.dram_tensor("xT", [D, S], F32R, kind="ExternalInput")
    wq = nc.dram_tensor("wq", [D, 512], F32R, kind="ExternalInput")
    wk = nc.dram_tensor("wk", [D, 128], F32R, kind="ExternalInput")
    wv = nc.dram_tensor("wv", [D, 128], F32R, kind="ExternalInput")
    wo = nc.dram_tensor("wo", [512, D], F32R, kind="ExternalInput")
    cos2 = nc.dram_tensor("cos2", [128, S], F32, kind="ExternalInput")
    sin2 = nc.dram_tensor("sin2", [128, S], F32, kind="ExternalInput")
    cmask = nc.dram_tensor("cmask", [128, 4, SB], F32, kind="ExternalInput")
    ident = nc.dram_tensor("ident", [128, 128], F32, kind="ExternalInput")
    y = nc.dram_tensor("y", [S, D], F32, kind="ExternalOutput")
    rscratch = nc.dram_tensor("rscratch", [NIT, 8, SB], F32)  # internal

    with tile.TileContext(nc) as tc:
        with (
            tc.tile_pool(name="persist", bufs=1) as persist,
            tc.tile_pool(name="consts", bufs=1) as consts,
        ):
            # ---- persistent SBUF tensors ----
            qT_sb = [persist.tile([128, S], F32R, name=f"qT{t}") for t in range(NET)]
            kT_sb = persist.tile([128, S], F32R, name="kT")
            v_ones0 = persist.tile([128, NJT, 65], F32R, name="v_ones0")
            v_ones1 = persist.tile([128, NJT, 65], F32R, name="v_ones1")
            outT = [persist.tile([128, S], F32R, name=f"outT{t}") for t in range(NET)]
            rowsum_all = persist.tile([8, NIT, SB], F32, name="rowsum_all")
            recip_all = persist.tile([8, NIT, SB], F32, name="recip_all")
            rowsum = [rowsum_all[:, b_, :] for b_ in range(NIT)]
            recip = [recip_all[:, b_, :] for b_ in range(NIT)]

            cos_sb = consts.tile([128, S], F32, name="cos_sb")
            sin_sb = consts.tile([128, S], F32, name="sin_sb")
            cmask_sb = consts.tile([128, 4, SB], F32, name="cmask_sb")
            ident_sb = consts.tile([128, 128], F32, name="ident_sb")
            ones_col = consts.tile([128, NJT, 1], F32, name="ones_col")
            nc.gpsimd.memset(ones_col[:], 1.0)
            nc.vector.tensor_copy(v_ones0[:, :, 64:65], ones_col[:])
            nc.vector.tensor_copy(v_ones1[:, :, 64:65], ones_col[:])

            # ================= Phase A: projections =================
            with (
                tc.tile_pool(name="wpool", bufs=1) as wpool,
                tc.tile_pool(name="xpool", bufs=3) as xpool,
                tc.tile_pool(name="apsum", bufs=1, space="PSUM") as apsum,
                tc.tile_pool(name="trpsum", bufs=2, space="PSUM") as trpsum,
                tc.tile_pool(name="atmp", bufs=3) as atmp,
            ):
                wq_sb = [wpool.tile([128, 512], F32R, name=f"wq{d}") for d in range(NDT)]
                wk_sb = [wpool.tile([128, 128], F32R, name=f"wk{d}") for d in range(NDT)]
                wv_sb = [wpool.tile([128, 128], F32R, name=f"wv{d}") for d in range(NDT)]

                for sb in range(NSB):
                    scol = slice(sb * SB, (sb + 1) * SB)
                    # six accumulating psum tiles live across the d loop
                    ps_q = [apsum.tile([128, SB], F32, name=f"psq{t}", tag=f"psq{t}")
                            for t in range(NET)]
                    ps_k = apsum.tile([128, SB], F32, name="psk", tag="psk")
                    ps_v = apsum.tile([128, SB], F32, name="psv", tag="psv")
                    for d in range(NDT):
                        if sb == 0:
                            # first-use weight loads, interleaved with the x stream
                            nc.sync.dma_start(wq_sb[d][:], wq[d * 128:(d + 1) * 128, :])
                            nc.sync.dma_start(wk_sb[d][:], wk[d * 128:(d + 1) * 128, :])
                            nc.sync.dma_start(wv_sb[d][:], wv[d * 128:(d + 1) * 128, :])
                            if d == 10:
                                nc.sync.dma_start(cos_sb[:], cos2[:])
                                nc.sync.dma_start(sin_sb[:], sin2[:])
                            if d == 14:
                                nc.sync.dma_start(ident_sb[:], ident[:])
                        if sb == 1 and d == 4:
                            nc.sync.dma_start(cmask_sb[:], cmask[:])
                        xt = xpool.tile([128, SB], F32R, name="xt", tag="xt")
                        nc.sync.dma_start(xt[:], xT[d * 128:(d + 1) * 128, scol])
                        st, sp = d == 0, d == NDT - 1
                        for t in range(NET):
                            nc.tensor.matmul(ps_q[t][:], _r(wq_sb[d][:, t * 128:(t + 1) * 128]),
                                             _r(xt[:]), start=st, stop=sp)
                        nc.tensor.matmul(ps_k[:], _r(wk_sb[d][:]), _r(xt[:]), start=st, stop=sp)
                        nc.tensor.matmul(ps_v[:], _r(wv_sb[d][:]), _r(xt[:]), start=st, stop=sp)

                    # v: copy psum -> sbuf, PE-transpose 128x128 blocks, split kv heads
                    vtmp = atmp.tile([128, SB], F32, name="vtmp", tag="vtmp")
                    nc.scalar.copy(vtmp[:], ps_v[:])
                    for u in range(SB // 128):
                        jt = sb * 4 + u
                        tr = trpsum.tile([128, 128], F32, name="tr", tag="tr")
                        nc.tensor.transpose(tr[:], vtmp[:, u * 128:(u + 1) * 128], ident_sb[:])
                        nc.vector.tensor_copy(v_ones0[:, jt, 0:64], tr[:, 0:64])
                        nc.vector.tensor_copy(v_ones1[:, jt, 0:64], tr[:, 64:128])

                    # RoPE, two passes: (1) drain all PSUM accumulators to
                    # SBUF on ACT so the banks free for the next s-block ASAP,
                    # (2) shift-DMA + mul/add chains on DVE.
                    rope_src = [(ps_k, kT_sb)] + [(ps_q[t], qT_sb[t]) for t in range(NET)]
                    qtmps = []
                    for ps, _dst in rope_src:
                        qtmp = atmp.tile([128, SB], F32, name="qtmp", tag="qtmp", bufs=4)
                        nc.scalar.copy(qtmp[:], ps[:])
                        qtmps.append(qtmp)
                    for qtmp, (_ps, dst) in zip(qtmps, rope_src):
                        rot = atmp.tile([128, SB], F32, name="rot", tag="rot")
                        for (a, b_) in ((0, 32), (32, 0), (64, 96), (96, 64)):
                            nc.gpsimd.dma_start(rot[b_:b_ + 32, :], qtmp[a:a + 32, :])
                        t1 = atmp.tile([128, SB], F32, name="t1", tag="t1")
                        nc.vector.tensor_mul(t1[:], qtmp[:], cos_sb[:, scol])
                        t2 = atmp.tile([128, SB], F32, name="t2", tag="t2")
                        nc.vector.tensor_mul(t2[:], rot[:], sin_sb[:, scol])
                        nc.vector.tensor_add(dst[:, scol], t1[:], t2[:])

            # ================= Phase B: attention =================
            # bi-outer so phase C (emitted later) can overlap later bi rounds.
            # Diagonal j-tiles only compute the valid column range [lo:512);
            # the triangular 128-strip is masked with cmask[:,0,0:128].
            with (
                tc.tile_pool(name="wopool", bufs=1) as wopool,
                tc.tile_pool(name="scpsum", bufs=2, space="PSUM") as scpsum,
                tc.tile_pool(name="pvpsum", bufs=1, space="PSUM") as pvpsum,
                tc.tile_pool(name="epool", bufs=4) as epool,
                tc.tile_pool(name="btmp", bufs=3) as btmp,
                tc.tile_pool(name="ypsum", bufs=2, space="PSUM") as ypsum,
                tc.tile_pool(name="ypool", bufs=3) as ypool,
            ):
                # prefetch Wo during attention, one f-tile per bi round
                wo_sb = [wopool.tile([128, D], F32R, name=f"wo{f}") for f in range(4)]

                tri = cmask_sb[:, 0, 0:128]
                for bi in range(NIT):
                    nc.sync.dma_start(wo_sb[bi][:], wo[bi * 128:(bi + 1) * 128, :])
                    icol = slice(bi * SB, (bi + 1) * SB)
                    for t in range(NET):
                        pvA = pvpsum.tile([65, SB], F32, name="pvA", tag="pvA")
                        pvB = pvpsum.tile([65, SB], F32, name="pvB", tag="pvB")
                        njt = 4 * bi + 4
                        for jt in range(njt):
                            jcol = slice(jt * 128, (jt + 1) * 128)
                            ro = jt - 4 * bi
                            lo = 128 * max(ro, 0)
                            iband = slice(bi * SB + lo, (bi + 1) * SB)
                            scA = scpsum.tile([128, SB], F32, name="scA", tag="scA")
                            scB = scpsum.tile([128, SB], F32, name="scB", tag="scB")
                            nc.tensor.matmul(scA[:, lo:], _r(kT_sb[0:64, jcol]),
                                             _r(qT_sb[t][0:64, iband]), start=True, stop=True)
                            nc.tensor.matmul(scB[:, lo:], _r(kT_sb[64:128, jcol]),
                                             _r(qT_sb[t][64:128, iband]), start=True, stop=True)
                            eA = epool.tile([128, SB], F32R, name="eA", tag="eA")
                            eB = epool.tile([128, SB], F32R, name="eB", tag="eB")
                            nc.scalar.activation(eA[:, lo:], scA[:, lo:],
                                                 mybir.ActivationFunctionType.Exp, scale=0.125)
                            nc.scalar.activation(eB[:, lo:], scB[:, lo:],
                                                 mybir.ActivationFunctionType.Exp, scale=0.125)
                            if ro >= 0:
                                nc.vector.tensor_mul(eA[:, lo:lo + 128], eA[:, lo:lo + 128], tri)
                                nc.vector.tensor_mul(eB[:, lo:lo + 128], eB[:, lo:lo + 128], tri)
                            st, sp = jt == 0, jt == njt - 1
                            nc.tensor.matmul(pvA[:, lo:], _r(v_ones0[:, jt, :]), _r(eA[:, lo:]),
                                             start=st, stop=sp)
                            nc.tensor.matmul(pvB[:, lo:], _r(v_ones1[:, jt, :]), _r(eB[:, lo:]),
                                             start=st, stop=sp)
                        # unnormalized outT + rowsum rows (2t, 2t+1) of this bi
                        nc.vector.tensor_copy(outT[t][0:64, icol], pvA[0:64, :])
                        pvsB = btmp.tile([65, SB], F32R, name="pvsB", tag="pvsB")
                        nc.vector.tensor_copy(pvsB[:], pvB[:])
                        nc.gpsimd.dma_start(outT[t][64:128, icol], pvsB[0:64, :])
                        rowA = btmp.tile([65, SB], F32, name="rowA", tag="rowA")
                        nc.vector.tensor_copy(rowA[64:65, :], pvA[64:65, :])
                        nc.gpsimd.dma_start(rowsum[bi][2 * t:2 * t + 1, :], rowA[64:65, :])
                        nc.gpsimd.dma_start(rowsum[bi][2 * t + 1:2 * t + 2, :],
                                            pvsB[64:65, :].bitcast(F32))

                    # per-bi normalization (unblocks phase C rows 4bi..4bi+4)
                    nc.vector.reciprocal(recip[bi][:], rowsum[bi][:])
                    nc.gpsimd.dma_start(rscratch[bi], recip[bi][:])
                    for t in range(NET):
                        bc = btmp.tile([128, SB], F32, name="bc", tag="bc")
                        nc.gpsimd.dma_start(
                            bc[0:64, :],
                            rscratch[bi, 2 * t:2 * t + 1, :].broadcast_to((64, SB)))
                        nc.gpsimd.dma_start(
                            bc[64:128, :],
                            rscratch[bi, 2 * t + 1:2 * t + 2, :].broadcast_to((64, SB)))
                        nc.vector.tensor_mul(outT[t][0:64, icol],
                                             outT[t][0:64, icol], bc[0:64, :])
                        nc.vector.tensor_mul(outT[t][64:128, icol],
                                             outT[t][64:128, icol], bc[64:128, :])

            # ================= Phase C: output projection =================
                for stt in range(S // 128):
                    srow = slice(stt * 128, (stt + 1) * 128)
                    for db in range(D // SB):
                        dcol = slice(db * SB, (db + 1) * SB)
                        yp = ypsum.tile([128, SB], F32, name="yp", tag="yp")
                        for f in range(4):
                            nc.tensor.matmul(yp[:], _r(outT[f][:, srow]),
                                             _r(wo_sb[f][:, dcol]),
                                             start=(f == 0), stop=(f == 3))
                        ys = ypool.tile([128, SB], F32, name="ys", tag="ys")
                        nc.scalar.copy(ys[:], yp[:])
                        nc.sync.dma_start(y[srow, dcol], ys[:])

    nc.compile()
    return nc


def host_inputs(x, Wq, Wk, Wv, Wo):
    """Per-core input maps (8 cores)."""
    inv = 1.0 / (10000.0 ** (np.arange(0, HD, 2, dtype=np.float64) / HD))
    freqs = np.outer(np.arange(S, dtype=np.float64), inv)          # [S, 32]
    emb = np.concatenate([freqs, freqs], axis=1)                   # [S, 64]
    cos = np.cos(emb).astype(np.float32)
    sin = np.sin(emb).astype(np.float32)
    cos2 = np.ascontiguousarray(np.tile(cos.T, (2, 1)))            # [128, S]
    sinf = np.concatenate([-sin[:, :32], sin[:, 32:]], axis=1)     # sign-folded
    sin2 = np.ascontiguousarray(np.tile(sinf.T, (2, 1)))
    j = np.arange(128)[:, None, None]
    ro = np.arange(4)[None, :, None]
    i = np.arange(SB)[None, None, :]
    cmask = (j + 128 * ro <= i).astype(np.float32)                 # [128, 4, 512]
    ident = np.eye(128, dtype=np.float32)

    Wq4 = Wq.reshape(D, H, HD)
    Wo4 = Wo.reshape(H, HD, D)
    Wk4 = Wk.reshape(D, KV, HD)
    Wv4 = Wv.reshape(D, KV, HD)

    maps = []
    for c in range(N_CORES):
        b, g2 = c // 4, c % 4
        gh = [8 * g2 + p for p in PERM]
        maps.append({
            "xT": np.ascontiguousarray(x[b].T),
            "wq": np.ascontiguousarray(Wq4[:, gh, :].reshape(D, 512)),
            "wk": np.ascontiguousarray(Wk4[:, [2 * g2, 2 * g2 + 1], :].reshape(D, 128)),
            "wv": np.ascontiguousarray(Wv4[:, [2 * g2, 2 * g2 + 1], :].reshape(D, 128)),
            "wo": np.ascontiguousarray(Wo4[gh].reshape(512, D)),
            "cos2": cos2, "sin2": sin2, "cmask": cmask, "ident": ident,
        })
    return maps


_NC_CACHE = None


def kernel(x, Wq, Wk, Wv, Wo):
    global LAST_RESULT, _NC_CACHE
    x = np.asarray(x, np.float32)
    maps = host_inputs(np.asarray(x, np.float32), np.asarray(Wq, np.float32),
                       np.asarray(Wk, np.float32), np.asarray(Wv, np.float32),
                       np.asarray(Wo, np.float32))
    if _NC_CACHE is None:
        _NC_CACHE = build_nc()
    trace = bool(os.environ.get("KERNEL_TRACE"))
    try:
        res = run_bass_kernel_spmd(_NC_CACHE, maps, list(range(N_CORES)), trace=trace)
    except (ImportError, ModuleNotFoundError):
        # NTFF profile hook unavailable in this environment — run untraced.
        res = run_bass_kernel_spmd(_NC_CACHE, maps, list(range(N_CORES)), trace=False)
    LAST_RESULT = res
    out = np.empty((B, S, D), np.float32)
    for b in range(B):
        out[b] = res.results[4 * b]["y"]
        for g2 in range(1, 4):
            out[b] += res.results[4 * b + g2]["y"]
    return out
